# revision 38
# baseline (speedup 1.0000x reference)
"""Trainium2 Bass kernel for nn_MedPoseAttention (multi-head cross-attention).

Full inputs in, full outputs out. Sharding: 8 cores = 4 batches x 2 query-row
halves. Each core computes one batch's K/V projections over the full context
(replicated within the pair) and attention + output projection for its 512
query rows, all 16 heads. No cross-core communication.

Per-core dataflow:
  Q/K/V/O projections in bf16 (weights + activations host-packed to bf16,
  fully-contiguous SBUF-layout DMAs).
  scores = k8.T @ q8 in fp8(e4m3) DoubleRow mode: contraction (p,2)-packed,
  kT8 zero-padded in the second k-slot, qT8 broadcast (stride-0) - 2x rate.
  exp on ScalarE -> bf16; PV with exp-block stationary, streaming [v|1]
  (F=65): out [q,65] accumulated over kv; denominator rides col 64.
  norm fused into PSUM read (reciprocal + tensor_scalar mult) -> bf16,
  PE-transposed back to [m,q] for the output projection.
  V bias folded into the output bias on host (bo2 = bv @ Wo + bo); O bias
  applied via a partition-broadcast add on the PSUM->SBUF copy.
"""

import sys

if "/opt/trn_rl_repo" not in sys.path:
    sys.path.insert(0, "/opt/trn_rl_repo")

import numpy as np
import ml_dtypes

import concourse.bass as bass  # noqa: F401
import concourse.mybir as mybir
from concourse import bacc, tile
from concourse.bass_utils import run_bass_kernel_spmd
from concourse.masks import make_identity

F32 = mybir.dt.float32
BF16 = mybir.dt.bfloat16
FP8 = mybir.dt.float8e4
MULT = mybir.AluOpType.mult
ADD = mybir.AluOpType.add
EXP = mybir.ActivationFunctionType.Exp
COPY = mybir.ActivationFunctionType.Copy
DR = mybir.MatmulPerfMode.DoubleRow

NPBF = ml_dtypes.bfloat16
NPE4 = ml_dtypes.float8_e4m3

B, L, D, H, HD = 4, 1024, 1024, 16, 64
NCORES = 8
LQ_C = 512  # query rows per core
NP = H // 2  # head pairs
SCALE = 0.125  # 1/sqrt(HD)

_PROGRAM = None


def build_program():
    nc = bacc.Bacc("TRN2", target_bir_lowering=False, debug=False, num_devices=NCORES)

    xq_d = nc.dram_tensor("xq_d", [128, 8 * LQ_C], BF16, kind="ExternalInput").ap()
    xc_d = nc.dram_tensor("xc_d", [128, 8 * L], BF16, kind="ExternalInput").ap()
    wq_d = nc.dram_tensor("wq_d", [128, 8192], BF16, kind="ExternalInput").ap()
    wk_d = nc.dram_tensor("wk_d", [128, 8192], BF16, kind="ExternalInput").ap()
    wv_d = nc.dram_tensor("wv_d", [128, 8192], BF16, kind="ExternalInput").ap()
    wo_d = nc.dram_tensor("wo_d", [128, 8192], BF16, kind="ExternalInput").ap()
    bq_d = nc.dram_tensor("bq_d", [128, NP], F32, kind="ExternalInput").ap()
    bk_d = nc.dram_tensor("bk_d", [128, NP], F32, kind="ExternalInput").ap()
    bo2_d = nc.dram_tensor("bo2_d", [1, D], F32, kind="ExternalInput").ap()

    out_d = nc.dram_tensor("out_d", [LQ_C, D], F32, kind="ExternalOutput").ap()
    res_d = nc.dram_tensor("res_d", [H * HD, LQ_C], BF16, kind="ExternalOutput").ap()

    xq_v = xq_d.rearrange("p (db j) -> p db j", db=8)
    # context is kv-major: [p, kvb, db, j] so V-proj can start on partial loads
    xc_v = xc_d.rearrange("p (kvb db j) -> p kvb db j", kvb=8, db=8)
    wv_v = wv_d.rearrange("p (db c) -> p db c", db=8)
    wo_v = wo_d.rearrange("p (hb c) -> p hb c", hb=8)

    with nc.allow_low_precision(reason="bf16/fp8 kernel"), tile.TileContext(nc) as tc:
        with (
            tc.tile_pool(name="persist", bufs=1) as persist,
            tc.tile_pool(name="wq_p", bufs=2) as wq_pool,
            tc.tile_pool(name="wk_p", bufs=2) as wk_pool,
            tc.tile_pool(name="wv_p", bufs=2) as wv_pool,
            tc.tile_pool(name="qt_p", bufs=2) as qt_pool,
            tc.tile_pool(name="qt8_p", bufs=2) as qt8_pool,
            tc.tile_pool(name="exp_p", bufs=18) as exp_pool,
            tc.tile_pool(name="small", bufs=2) as small,
            tc.tile_pool(name="psP", bufs=1, space="PSUM") as psP,
            tc.tile_pool(name="psS", bufs=2, space="PSUM") as psS,
            tc.tile_pool(name="psV", bufs=2, space="PSUM") as psV,
        ):
            # ---- persistent tiles ----
            xq_all = persist.tile([128, 8, LQ_C], BF16, tag="xq", name="xq_all")
            xc_all = persist.tile([128, 8, 8, 128], BF16, tag="xc", name="xc_all")
            v_all = persist.tile([128, 128, 65], BF16, tag="vb", name="v_all")
            k8 = [
                persist.tile([128, 2, L], FP8, tag=f"k8{i}", name=f"k8{i}")
                for i in range(2)
            ]
            ident = persist.tile([128, 128], BF16, tag="id", name="ident")
            bo_b = persist.tile([128, D], F32, tag="bo_b", name="bo_b")
            bo2_sb = persist.tile([1, D], F32, tag="bo2", name="bo2_sb")
            bq_sb = persist.tile([128, NP], F32, tag="bq", name="bq_sb")
            bk_sb = persist.tile([128, NP], F32, tag="bk", name="bk_sb")
            mt = [
                persist.tile([128, LQ_C], BF16, tag=f"mt{p}", name=f"mt{p}")
                for p in range(NP)
            ]
            wo_t = [
                persist.tile([128, 8, 512], BF16, tag=f"wo{c}", name=f"wo{c}")
                for c in range(2)
            ]

            qT = [None] * NP
            qT8 = [None] * NP
            wq_t = [None] * NP
            wk_t = [None] * NP
            wv_t = [None] * 4
            ets = [[None] * 8 for _ in range(NP)]

            def emit_preamble():
                # first-needed first; the shared DMA pool serializes transfers
                # in roughly this order
                wq_t[0] = wq_pool.tile([128, 8, 128], BF16, tag="wq", name="wq0")
                nc.sync.dma_start(wq_t[0][:], wq_d[:, 0:1024].rearrange("p (db m) -> p db m", db=8))
                nc.scalar.dma_start(xq_all[:, 0:4, :], xq_v[:, 0:4, :])
                wv_t[0] = wv_pool.tile([128, 8, 256], BF16, tag="wv", name="wv0")
                nc.sync.dma_start(wv_t[0][:], wv_v[:, :, 0:256])
                nc.sync.dma_start(xc_all[:, 0, :, :], xc_v[:, 0, :, :])
                wk_t[0] = wk_pool.tile([128, 8, 128], BF16, tag="wk", name="wk0")
                nc.sync.dma_start(wk_t[0][:], wk_d[:, 0:1024].rearrange("p (db m) -> p db m", db=8))
                nc.scalar.dma_start(xc_all[:, 1, :, :], xc_v[:, 1, :, :])
                nc.scalar.dma_start(xq_all[:, 4:8, :], xq_v[:, 4:8, :])
                nc.sync.dma_start(xc_all[:, 2, :, :], xc_v[:, 2, :, :])
                nc.scalar.dma_start(xc_all[:, 3, :, :], xc_v[:, 3, :, :])
                nc.sync.dma_start(xc_all[:, 4, :, :], xc_v[:, 4, :, :])
                nc.scalar.dma_start(xc_all[:, 5, :, :], xc_v[:, 5, :, :])
                nc.sync.dma_start(xc_all[:, 6, :, :], xc_v[:, 6, :, :])
                nc.scalar.dma_start(xc_all[:, 7, :, :], xc_v[:, 7, :, :])
                nc.vector.memset(k8[0][:, 1, :], 0.0)
                nc.vector.memset(k8[1][:, 1, :], 0.0)
                nc.vector.memset(v_all[:, :, 64:65], 1.0)
                make_identity(nc, ident[:])

            def emit_late_consts():
                nc.scalar.dma_start(bq_sb[:], bq_d[:])
                nc.scalar.dma_start(bk_sb[:], bk_d[:])
                nc.scalar.dma_start(bo2_sb[:], bo2_d[:])
                nc.gpsimd.partition_broadcast(bo_b[:], bo2_sb[0:1, :])

            def finish_qproj(pr, qps):
                qT[pr] = qt_pool.tile([128, LQ_C], BF16, tag="qt", name=f"qt{pr}")
                nc.vector.tensor_scalar_add(qT[pr][:], qps[:], bq_sb[:, pr : pr + 1])
                qT8[pr] = qt8_pool.tile([128, LQ_C], FP8, tag="qt8", name=f"qt8{pr}")
                nc.scalar.activation(qT8[pr][:], qT[pr][:], COPY)
                nc.sync.dma_start(res_d[pr * 128 : (pr + 1) * 128, :], qT[pr][:])

            def emit_vgroup(ch, kvb):
                vps = psS.tile([128, 256], F32, tag="sps", name=f"vps{ch}{kvb}")
                for d in range(8):
                    nc.tensor.matmul(
                        vps[:],
                        lhsT=xc_all[:, kvb, d, :],
                        rhs=wv_t[ch][:, d, :],
                        start=(d == 0), stop=(d == 7),
                    )
                nc.vector.tensor_copy(
                    v_all[:, kvb * 16 + ch * 4 : kvb * 16 + ch * 4 + 4, 0:64],
                    vps[:].rearrange("p (h m) -> p h m", h=4),
                )

            def emit_kchunk0(ch):
                kps = psP.tile([128, 512], F32, tag="kps", name=f"kps0{ch}")
                for d in range(8):
                    nc.tensor.matmul(
                        kps[:], lhsT=wk_t[0][:, d, :],
                        rhs=xc_all[:, ch * 4 : (ch + 1) * 4, d, :],
                        start=(d == 0), stop=(d == 7),
                    )
                nc.vector.tensor_scalar_add(
                    k8[0][:, 0, ch * 512 : (ch + 1) * 512], kps[:], bk_sb[:, 0:1]
                )

            def emit_startup():
                # emission order tracks DMA chunk arrivals to keep PE fed
                qps = psP.tile([128, LQ_C], F32, tag="qps", name="qps0")
                for d in range(4):
                    nc.tensor.matmul(
                        qps[:], lhsT=wq_t[0][:, d, :], rhs=xq_all[:, d, :],
                        start=(d == 0), stop=False,
                    )
                emit_vgroup(0, 0)
                emit_vgroup(0, 1)
                for d in range(4, 8):
                    nc.tensor.matmul(
                        qps[:], lhsT=wq_t[0][:, d, :], rhs=xq_all[:, d, :],
                        start=False, stop=(d == 7),
                    )
                emit_vgroup(0, 2)
                finish_qproj(0, qps)
                emit_vgroup(0, 3)
                emit_kchunk0(0)
                # remaining wv loads, queued behind xc
                for c in range(1, 4):
                    wv_t[c] = wv_pool.tile([128, 8, 256], BF16, tag="wv", name=f"wv{c}")
                    nc.sync.dma_start(wv_t[c][:], wv_v[:, :, c * 256 : (c + 1) * 256])
                emit_vgroup(0, 4)
                emit_vgroup(0, 5)
                emit_vgroup(0, 6)
                emit_vgroup(0, 7)
                emit_kchunk0(1)
                for ch in range(1, 4):
                    for kvb in range(8):
                        emit_vgroup(ch, kvb)

            # deferred-transpose state: (pair, h, qb, pvn_tile)
            pending = [None]

            def emit_pv_group(hp, h, qb, use_act=False):
                hg0 = 2 * hp + h
                pv = psV.tile([128, 65], F32, tag="pvmt", name=f"pv{hp}{h}{qb}")
                for kvb in range(8):
                    nc.tensor.matmul(
                        pv[:],
                        lhsT=ets[hp][kvb][:, h * 512 + qb * 128 : h * 512 + (qb + 1) * 128],
                        rhs=v_all[:, kvb * 16 + hg0, :],
                        start=(kvb == 0), stop=(kvb == 7),
                    )
                rcp = small.tile([128, 1], F32, tag="rcp", name=f"rc{hp}{h}{qb}")
                nc.vector.reciprocal(rcp[:], pv[:, 64:65])
                pvn = small.tile([128, 64], BF16, tag="pvn", name=f"pn{hp}{h}{qb}")
                if use_act:
                    # tail only: ScalarE is idle there, DVE is the tail bound
                    nc.scalar.activation(pvn[:], pv[:, 0:64], COPY, scale=rcp[:])
                else:
                    nc.vector.tensor_scalar_mul(pvn[:], pv[:, 0:64], rcp[:])
                return (hp, h, qb, pvn)

            def emit_pv_transpose(entry, use_act=False):
                hp, h, qb, pvn = entry
                mtp = psV.tile([128, 128], BF16, tag="pvmt", name=f"mp{hp}{h}{qb}")
                nc.tensor.transpose(
                    mtp[h * 64 : (h + 1) * 64, :], pvn[:], ident[:],
                    tile_position=(0, h * 64),
                )
                dst = mt[hp][h * 64 : (h + 1) * 64, qb * 128 : (qb + 1) * 128]
                src = mtp[h * 64 : (h + 1) * 64, :]
                if use_act:
                    nc.scalar.activation(dst, src, COPY)
                else:
                    nc.vector.tensor_copy(dst, src)

            osb_t = [None] * 8  # O-proj partial sums parked in SBUF

            def emit_obegin(r):
                # first 6 hcb of O block r=(qb,ch), interleaved into pair-7;
                # partial + output bias parked in SBUF
                qb, ch = divmod(r, 2)
                ops = psP.tile(
                    [128, 512], F32, tag=("qps" if r % 2 == 0 else "kps"),
                    name=f"opsb{qb}{ch}",
                )
                for hcb in range(6):
                    nc.tensor.matmul(
                        ops[:],
                        lhsT=mt[hcb][:, qb * 128 : (qb + 1) * 128],
                        rhs=wo_t[ch][:, hcb, :],
                        start=(hcb == 0), stop=(hcb == 5),
                    )
                osb_t[r] = small.tile(
                    [128, 512], F32, tag="osb", name=f"ob{qb}{ch}", bufs=8
                )
                nc.vector.tensor_tensor(
                    osb_t[r][:], ops[:], bo_b[:, ch * 512 : (ch + 1) * 512], op=ADD
                )

            def emit_ofinish(r, split=False):
                qb, ch = divmod(r, 2)
                ops = psP.tile(
                    [128, 512], F32, tag=("qps" if r % 2 == 0 else "kps"),
                    name=f"opsf{qb}{ch}",
                )
                for hcb in (6, 7):
                    nc.tensor.matmul(
                        ops[:],
                        lhsT=mt[hcb][:, qb * 128 : (qb + 1) * 128],
                        rhs=wo_t[ch][:, hcb, :],
                        start=(hcb == 6), stop=(hcb == 7),
                    )
                # split: pipeline the PSUM-add with the store on the last blocks
                for c0, c1 in ([(0, 256), (256, 512)] if split else [(0, 512)]):
                    nc.vector.tensor_tensor(
                        osb_t[r][:, c0:c1], osb_t[r][:, c0:c1], ops[:, c0:c1], op=ADD
                    )
                    nc.sync.dma_start(
                        out_d[qb * 128 : (qb + 1) * 128, ch * 512 + c0 : ch * 512 + c1],
                        osb_t[r][:, c0:c1],
                    )

            def emit_pair_loop(p):
                nxt = p + 1 if p + 1 < NP else None
                k8cur = k8[p % 2]
                if nxt is not None:
                    wq_t[nxt] = wq_pool.tile([128, 8, 128], BF16, tag="wq", name=f"wq{nxt}")
                    nc.sync.dma_start(
                        wq_t[nxt][:],
                        wq_d[:, nxt * 1024 : (nxt + 1) * 1024].rearrange("p (db m) -> p db m", db=8),
                    )
                    wk_t[nxt] = wk_pool.tile([128, 8, 128], BF16, tag="wk", name=f"wk{nxt}")
                    nc.sync.dma_start(
                        wk_t[nxt][:],
                        wk_d[:, nxt * 1024 : (nxt + 1) * 1024].rearrange("p (db m) -> p db m", db=8),
                    )
                    qps = psP.tile([128, LQ_C], F32, tag="qps", name=f"qps{nxt}")
                    k8n = k8[nxt % 2]
                    kps = None
                for s in range(8):
                    # scores (p, s): 4 fp8 DoubleRow matmuls
                    sps = psS.tile([128, 1024], F32, tag="sps", name=f"sps{p}{s}")
                    for h in range(2):
                        for qc in range(2):
                            rhs = (
                                qT8[p][h * 64 : (h + 1) * 64, qc * 256 : (qc + 1) * 256]
                                .unsqueeze(1)
                                .broadcast_to((64, 2, 256))
                            )
                            nc.tensor.matmul(
                                sps[:, h * 512 + qc * 256 : h * 512 + (qc + 1) * 256],
                                lhsT=k8cur[h * 64 : (h + 1) * 64, :, s * 128 : (s + 1) * 128],
                                rhs=rhs,
                                start=True, stop=True,
                                perf_mode=DR,
                            )
                    et = exp_pool.tile([128, 1024], BF16, tag="et", name=f"et{p}{s}")
                    nc.scalar.activation(et[:], sps[:], EXP, scale=SCALE)
                    ets[p][s] = et
                    # deferred transpose + PV group for pair p-1
                    if pending[0] is not None:
                        emit_pv_transpose(pending[0])
                        pending[0] = None
                    if p > 0:
                        pending[0] = emit_pv_group(p - 1, s % 2, s // 2)
                    if p == NP - 1:
                        emit_obegin(s)
                    # next-pair projections
                    if nxt is not None:
                        nc.tensor.matmul(
                            qps[:], lhsT=wq_t[nxt][:, s, :], rhs=xq_all[:, s, :],
                            start=(s == 0), stop=(s == 7),
                        )
                        ch, d0 = divmod(2 * s, 8)
                        if d0 == 0:
                            kps = psP.tile([128, 512], F32, tag="kps", name=f"kps{nxt}{ch}")
                        for d in (d0, d0 + 1):
                            nc.tensor.matmul(
                                kps[:], lhsT=wk_t[nxt][:, d, :],
                                rhs=xc_all[:, ch * 4 : (ch + 1) * 4, d, :],
                                start=(d == 0), stop=(d == 7),
                            )
                        if d0 + 1 == 7:
                            nc.vector.tensor_scalar_add(
                                k8n[:, 0, ch * 512 : (ch + 1) * 512], kps[:],
                                bk_sb[:, nxt : nxt + 1],
                            )
                if nxt is not None:
                    finish_qproj(nxt, qps)

            def emit_tail():
                # PV groups of pair 7 (qb-major) interleaved with the O-block
                # finishers (hcb 6,7 + SBUF accumulate + store)
                for qb in range(4):
                    for h in range(2):
                        if pending[0] is not None:
                            emit_pv_transpose(pending[0], use_act=True)
                        pending[0] = emit_pv_group(7, h, qb, use_act=True)
                    if qb > 0:
                        emit_ofinish(2 * (qb - 1))
                        emit_ofinish(2 * (qb - 1) + 1)
                emit_pv_transpose(pending[0], use_act=True)
                pending[0] = None
                emit_ofinish(6, split=True)
                emit_ofinish(7, split=True)

            emit_preamble()
            emit_late_consts()
            emit_startup()
            for p in range(NP):
                if p == 5:
                    for c in range(2):
                        nc.sync.dma_start(
                            wo_t[c][:], wo_v[:, :, c * 512 : (c + 1) * 512]
                        )
                emit_pair_loop(p)
            emit_tail()

    nc.compile()
    return nc


def _marshal(inputs):
    q = np.asarray(inputs["queries"], dtype=np.float32)
    c = np.asarray(inputs["context"], dtype=np.float32)
    Wq = np.asarray(inputs["Wq"], dtype=np.float32)
    Wk = np.asarray(inputs["Wk"], dtype=np.float32)
    Wv = np.asarray(inputs["Wv"], dtype=np.float32)
    Wo = np.asarray(inputs["Wo"], dtype=np.float32)
    bq = np.asarray(inputs["bq"], dtype=np.float32)
    bk = np.asarray(inputs["bk"], dtype=np.float32)
    bv = np.asarray(inputs["bv"], dtype=np.float32)
    bo = np.asarray(inputs["bo"], dtype=np.float32)

    def pack_w(W):  # [H, D, HD] -> [128, 8192] bf16 (p, pr, db, m)
        Wt = W.transpose(1, 0, 2).reshape(D, H * HD)
        return np.ascontiguousarray(
            Wt.reshape(8, 128, 8, 128).transpose(1, 2, 0, 3).reshape(128, 8192)
        ).astype(NPBF)

    def pack_rows(Wt):  # [D(rows=8*128), C] -> [128, 8*C]
        C = Wt.shape[1]
        return np.ascontiguousarray(
            Wt.reshape(8, 128, C).transpose(1, 0, 2).reshape(128, 8 * C)
        ).astype(NPBF)

    wq_pk = pack_w(Wq)
    wk_pk = pack_w(Wk)
    wv_pk = pack_rows(Wv.transpose(1, 0, 2).reshape(D, H * HD))
    wo_pk = pack_rows(Wo)

    bq_c = np.ascontiguousarray(bq.reshape(NP, 128).T)
    bk_c = np.ascontiguousarray(bk.reshape(NP, 128).T)
    bo2 = (
        bv.reshape(1, H * HD).astype(np.float64) @ Wo.astype(np.float64)
        + bo.astype(np.float64)
    ).astype(np.float32)

    shared = {
        "wq_d": wq_pk, "wk_d": wk_pk, "wv_d": wv_pk, "wo_d": wo_pk,
        "bq_d": bq_c, "bk_d": bk_c, "bo2_d": bo2,
    }
    in_maps = []
    for core in range(NCORES):
        b, half = core // 2, core % 2
        m = dict(shared)
        xq = q[b].T[:, half * LQ_C : (half + 1) * LQ_C]
        m["xq_d"] = np.ascontiguousarray(
            xq.reshape(8, 128, LQ_C).transpose(1, 0, 2).reshape(128, 8 * LQ_C)
        ).astype(NPBF)
        # kv-major: [p, kvb, db, j]
        m["xc_d"] = np.ascontiguousarray(
            c[b].T.reshape(8, 128, 8, 128).transpose(1, 2, 0, 3).reshape(128, 8192)
        ).astype(NPBF)
        in_maps.append(m)
    return in_maps


def kernel(**inputs):
    global _PROGRAM
    if _PROGRAM is None:
        _PROGRAM = build_program()
    in_maps = _marshal(inputs)
    res = run_bass_kernel_spmd(_PROGRAM, in_maps, list(range(NCORES)))
    out = np.empty((B, L, D), np.float32)
    residual = np.empty((B, L, H * HD), np.float32)
    for core in range(NCORES):
        b, half = core // 2, core % 2
        sl = slice(half * LQ_C, (half + 1) * LQ_C)
        out[b, sl, :] = res.results[core]["out_d"]
        residual[b, sl, :] = res.results[core]["res_d"].astype(np.float32).T
    return out, residual
